# revision 33
# baseline (speedup 1.0000x reference)
"""Dual-RoPE attention block (B=8, S=1024, 16 heads x 64) on 8 NeuronCores.

Sharding: data-parallel over batch, one batch element per core.

Software-pipelined single-pass structure (v2): the per-head-pair attention
stream (scores -> exp -> PV) is ACT-paced; projection matmuls (QKV, woT
loads) are emitted as PE "filler" generators inside the attention stream so
the tensor engine never idles while the scalar engine exponentiates.

PSUM plan (8 banks): tag "scp" = 2 x [128,1024] f32 (4 banks), shared by
score tiles, projection n-group accumulators and o_proj accumulators -- every
allocation is a complete bank-group whose reader is emitted within the same
filler piece, so pool rotation WAR deps sequence reuse correctly.  Tag
"pvps" = 2 x [65,1024] f32 (4 banks): the inline head (g0) of the current
pass ping-pongs against the deferred second head (g1) sweep that overlaps
the next pass's units.

Numerics identical to v1 baseline: all matmuls bf16 in / fp32 PSUM, softmax
without max-subtraction, 1/sum via fast reciprocal of PV row 0
(vext col0 = 2.0 so rec = 0.5/sum includes the two-pass averaging).
"""

import numpy as np
import ml_dtypes
from collections import deque

B, S, DM = 8, 1024, 1024
NH, HD = 16, 64
HD1 = HD + 1
NC = 8                # cores

_CACHE = {}


def _build():
    key = "v2"
    if key in _CACHE:
        return _CACHE[key]
    from concourse import bacc, mybir
    import concourse.tile as tile

    f32 = mybir.dt.float32
    bf16 = mybir.dt.bfloat16
    EXP = mybir.ActivationFunctionType.Exp

    nc = bacc.Bacc("TRN2", target_bir_lowering=False, debug=False,
                   enable_asserts=False, num_devices=NC)

    xT_d = nc.dram_tensor("xT", [DM, S], bf16, kind="ExternalInput").ap()
    wqkr_d = nc.dram_tensor("wqkr", [16, 128, DM], bf16,
                            kind="ExternalInput").ap()
    wvT_d = nc.dram_tensor("wvT", [DM, DM], bf16, kind="ExternalInput").ap()
    woT_d = nc.dram_tensor("woT", [DM, DM], bf16, kind="ExternalInput").ap()
    trigC_d = nc.dram_tensor("trigC", [2, 128, S], bf16,
                             kind="ExternalInput").ap()
    trigS_d = nc.dram_tensor("trigS", [2, 128, S], bf16,
                             kind="ExternalInput").ap()
    out_d = nc.dram_tensor("out", [S, DM], f32, kind="ExternalOutput").ap()

    with tile.TileContext(nc) as tc:
        with (
            tc.tile_pool(name="sb", bufs=1) as sb,
            tc.tile_pool(name="ps", bufs=1, space="PSUM") as ps,
        ):
            # ---------------- persistent SBUF tiles + loads ----------------
            # Critical-path loads (first projection) go first on the sync
            # queue; everything else issues from the gpsimd queue so the
            # ~600ns-per-DMA descriptor-gen serialization doesn't delay the
            # first matmul.
            wqk0 = {}
            for t in (0, 8):
                w = sb.tile([128, DM], bf16, tag="wqk", bufs=6,
                            name=f"wqk{t}")
                nc.scalar.dma_start(w[:], wqkr_d[t])
                wqk0[t] = w

            xT_sb = [sb.tile([128, S], bf16, tag="xT", bufs=8, name=f"xT{i}")
                     for i in range(8)]
            for i in range(8):
                eng = nc.sync if i % 2 == 0 else nc.scalar
                eng.dma_start(xT_sb[i][:], xT_d[i * 128:(i + 1) * 128, :])

            trigC_t = [sb.tile([128, S], bf16, tag="trig", bufs=4,
                               name=f"trigC{p}") for p in range(2)]
            trigS_t = [sb.tile([128, S], bf16, tag="trig", bufs=4,
                               name=f"trigS{p}") for p in range(2)]
            for p in range(2):
                nc.sync.dma_start(trigC_t[p][:], trigC_d[p])
                nc.sync.dma_start(trigS_t[p][:], trigS_d[p])

            wvT_sb = [sb.tile([128, DM], bf16, tag="wv", bufs=8,
                              name=f"wv{i}") for i in range(8)]
            for i in range(8):
                nc.gpsimd.dma_start(wvT_sb[i][:],
                                    wvT_d[i * 128:(i + 1) * 128, :])

            ones65 = sb.tile([1, HD1], bf16, tag="ones", bufs=1,
                             name="ones65")
            nc.vector.memset(ones65[0:1, :], 1.0)

            vext = [sb.tile([128, NH * HD1], bf16, tag="vext", bufs=8,
                            name=f"vext{i}") for i in range(8)]
            attn_b = [sb.tile([128, S], bf16, tag="attnb", bufs=8,
                              name=f"attnb{i}") for i in range(8)]

            roped = {}   # (pss, t) -> tile; t: 0..7 q chunk, 8..15 k chunk
            cts = {}     # (pss, g) -> ct tile for the current cc
            woT_sb = []

            # ------------------- emission helper machinery -----------------
            fillers = deque()   # generators; each yield ~= one 8-mm piece

            def emit_filler(n=1):
                for _ in range(n):
                    while fillers:
                        try:
                            next(fillers[0])
                            break
                        except StopIteration:
                            fillers.popleft()
                    else:
                        return

            def drain(gen):
                for _ in gen:
                    pass

            def drain_fillers():
                while fillers:
                    drain(fillers.popleft())

            # ------------------------- generators --------------------------
            def gen_proj_qk(cc):
                """Project q (t=cc) and k (t=8+cc) into [c, s] layout, then
                RoPE: both chunks' matmul groups first, then pass-0 RoPE for
                both (DVE work pipelines under the k-chunk matmuls), pass-1
                RoPE last (consumed as early filler)."""
                chunks = (cc, 8 + cc)
                qks, sws = {}, {}
                for t in chunks:
                    if t in wqk0:
                        w = wqk0[t]
                    else:
                        w = sb.tile([128, DM], bf16, tag="wqk", bufs=6,
                                    name=f"wqk{t}")
                        nc.sync.dma_start(w[:], wqkr_d[t])
                    qk = sb.tile([128, S], bf16, tag="qk", bufs=2,
                                 name=f"qk{t}")
                    for n in range(2):
                        pst = ps.tile([128, S], f32, tag="scp", bufs=2)
                        for dc in range(8):
                            nc.tensor.matmul(
                                pst[:, 0:512],
                                w[:, dc * 128:(dc + 1) * 128],
                                xT_sb[dc][:, n * 512:(n + 1) * 512],
                                start=(dc == 0), stop=(dc == 7))
                        nc.vector.tensor_copy(qk[:, n * 512:(n + 1) * 512],
                                              pst[:, 0:512])
                        yield
                    sw = sb.tile([128, S], bf16, tag="sw", bufs=2,
                                 name=f"sw{t}")
                    for hh in range(2):
                        for f in range(2):
                            o0 = hh * 64 + f * 32
                            i0 = hh * 64 + (1 - f) * 32
                            nc.sync.dma_start(sw[o0:o0 + 32, :],
                                              qk[i0:i0 + 32, :])
                    qks[t], sws[t] = qk, sw
                for pss in range(2):
                    for t in chunks:
                        a = sb.tile([128, S], bf16, tag="ropeA", bufs=2,
                                    name=f"ropeA{t}_{pss}")
                        bb = sb.tile([128, S], bf16, tag="ropeB", bufs=2,
                                     name=f"ropeB{t}_{pss}")
                        r = sb.tile([128, S], bf16, tag="roped", bufs=12,
                                    name=f"rope{pss}_{t}")
                        nc.vector.tensor_mul(a[:], qks[t][:],
                                             trigC_t[pss][:])
                        nc.vector.tensor_mul(bb[:], sws[t][:],
                                             trigS_t[pss][:])
                        nc.vector.tensor_add(r[:], a[:], bb[:])
                        roped[(pss, t)] = r
                        yield

            def gen_proj_v():
                """V chunks in [s, c] row layout, strided into vext with 2.0
                in column 0 of each head block."""
                for sc in range(8):
                    vv = vext[sc][:].rearrange("p (h e) -> p h e", e=HD1)
                    nc.vector.memset(vv[:, :, 0:1], 2.0)
                    for n in range(2):
                        pst = ps.tile([128, S], f32, tag="scp", bufs=2)
                        for dc in range(8):
                            nc.tensor.matmul(
                                pst[:, 0:512],
                                xT_sb[dc][:, sc * 128:(sc + 1) * 128],
                                wvT_sb[dc][:, n * 512:(n + 1) * 512],
                                start=(dc == 0), stop=(dc == 7))
                        nc.vector.tensor_copy(
                            vv[:, 8 * n:8 * n + 8, 1:HD1],
                            pst[:, 0:512].rearrange("p (h e) -> p h e", e=HD))
                        yield

            def gen_wo():
                # woT reuses the wv SBUF slots; all V-proj reads of wvT are
                # emitted before this generator is reached in the queue, so
                # WAR deps sequence the overwrite correctly.
                for i in range(8):
                    w = sb.tile([128, DM], bf16, tag="wv", bufs=8,
                                name=f"wo{i}")
                    nc.sync.dma_start(w[:], woT_d[i * 128:(i + 1) * 128, :])
                    woT_sb.append(w)
                    if i % 4 == 3:
                        yield

            def norm(cc, pss, g, pvp):
                """ct = pv * (1/(2*sum)) broadcast across the 65 rows."""
                h = 2 * cc + g
                recf = sb.tile([1, S], f32, tag="recf", bufs=1,
                               name=f"recf{pss}_{h}")
                nc.vector.reciprocal_approx_fast(recf[0:1, :], pvp[0:1, :])
                rec = sb.tile([1, S], bf16, tag="rec", bufs=2,
                              name=f"rec{pss}_{h}")
                with nc.allow_low_precision(reason="bf16 recip of sums"):
                    nc.vector.tensor_copy(rec[0:1, :], recf[0:1, :])
                bc = sb.tile([HD1, S], bf16, tag="bc", bufs=2,
                             name=f"bc{pss}_{h}")
                nc.gpsimd.partition_broadcast(bc[:, :], rec[0:1, :],
                                              channels=HD1)
                # pv_sb cast runs on DVE while the broadcast runs on Pool
                pv_sb = sb.tile([HD1, S], bf16, tag="pvsb", bufs=4,
                                name=f"pvsb{pss}_{h}")
                nc.vector.tensor_copy(pv_sb[:], pvp[:])
                ct = sb.tile([HD1, S], bf16, tag="ct", bufs=4,
                             name=f"ct{pss}_{h}")
                nc.vector.tensor_mul(ct[:], pv_sb[:], bc[:])
                cts[(pss, g)] = ct

            def combine(cc, g):
                ah = sb.tile([HD1, S], bf16, tag="ah", bufs=2, name=f"ah{g}")
                nc.vector.tensor_add(ah[:], cts[(0, g)][:], cts[(1, g)][:])
                eng = nc.sync if g == 0 else nc.scalar
                eng.dma_start(attn_b[cc][g * 64:(g + 1) * 64, :],
                              ah[1:HD1, :])

            def gen_pv_sweep(cc, pss, g, ess, do_combine):
                """Deferred PV sweep for head g of (cc, pss), then its norm
                (and the head-pair combine when this is the last piece)."""
                h = 2 * cc + g
                pvp = ps.tile([HD1, S], f32, tag="pvps", bufs=2)
                for kc in range(8):
                    for n in range(2):
                        nc.tensor.matmul(
                            pvp[:, n * 512:(n + 1) * 512],
                            vext[kc][:, h * HD1:(h + 1) * HD1],
                            ess[(kc, n)][:, g * 512:(g + 1) * 512],
                            start=(kc == 0), stop=(kc == 7))
                    if kc % 2 == 1:
                        yield
                norm(cc, pss, g, pvp)
                if do_combine:
                    combine(cc, g)
                yield

            # --------------------------- schedule ---------------------------
            # Lead-in: head-pair 0's four matmul groups + pass-0 RoPE run
            # inline (nothing else in the DVE queue so the rope chain fires
            # as the groups complete); pass-1 RoPE pieces lead the filler
            # queue, followed by V-proj and woT loads.
            g0 = gen_proj_qk(0)
            for _ in range(6):
                next(g0)
            fillers.append(g0)
            v_gen = gen_proj_v()
            fillers.append(v_gen)
            fillers.append(gen_wo())

            prev_g1 = None
            for cc in range(8):
                hE = 2 * cc
                qk_next = None
                if cc < 7:
                    qk_next = gen_proj_qk(cc + 1)
                    fillers.append(qk_next)
                for pss in range(2):
                    while (pss, cc) not in roped or (pss, 8 + cc) not in roped:
                        emit_filler(1)
                    q1 = {pss: roped[(pss, cc)]}
                    k1 = {pss: roped[(pss, 8 + cc)]}
                    defer_g0 = (cc == 0 and pss == 0)
                    if defer_g0:
                        # put a few V pieces ahead of the first scores in the
                        # PE queue: they execute while the RoPE chain finishes
                        emit_filler(6)
                    inline_g1 = (cc == 7 and pss == 1)
                    ess = {}
                    if inline_g1 and prev_g1 is not None:
                        # Pre-drain the previous deferred sweep so the final
                        # head-pair's tail chain is as short as possible.
                        drain(prev_g1)
                        prev_g1 = None
                    if not defer_g0:
                        pvp0 = ps.tile([HD1, S], f32, tag="pvps", bufs=2)
                    if inline_g1:
                        pvp1 = ps.tile([HD1, S], f32, tag="pvps", bufs=2)
                    for kc in range(8):
                        for n in range(2):
                            scp = ps.tile([128, S], f32, tag="scp", bufs=2)
                            for g, hh in ((0, 0), (1, 64)):
                                nc.tensor.matmul(
                                    scp[:, g * 512:(g + 1) * 512],
                                    k1[pss][hh:hh + 64,
                                            kc * 128:(kc + 1) * 128],
                                    q1[pss][hh:hh + 64,
                                            n * 512:(n + 1) * 512],
                                    start=True, stop=True)
                            es = sb.tile([128, S], bf16, tag="es", bufs=20,
                                         name=f"es{pss}_{cc}_{kc}_{n}")
                            nc.scalar.activation(es[:], scp[:], EXP,
                                                 scale=0.125)
                            ess[(kc, n)] = es
                            emit_filler(1)
                            if not defer_g0:
                                nc.tensor.matmul(
                                    pvp0[:, n * 512:(n + 1) * 512],
                                    vext[kc][:, hE * HD1:(hE + 1) * HD1],
                                    es[:, 0:512],
                                    start=(kc == 0), stop=(kc == 7))
                            if inline_g1:
                                nc.tensor.matmul(
                                    pvp1[:, n * 512:(n + 1) * 512],
                                    vext[kc][:, (hE + 1) * HD1:
                                               (hE + 2) * HD1],
                                    es[:, 512:1024],
                                    start=(kc == 0), stop=(kc == 7))
                    if defer_g0:
                        # vext must be fully projected before any PV of
                        # head-pair 0; drain V then run g0's sweep inline.
                        drain(v_gen)
                        drain(gen_pv_sweep(cc, pss, 0, ess, False))
                    elif inline_g1:
                        # final head-pair: interleave the two norm chains for
                        # minimum latency before o_proj can start
                        pair = ((0, pvp0), (1, pvp1))
                        recs = {}
                        for g, pvp in pair:
                            recf = sb.tile([1, S], f32, tag="recf", bufs=1,
                                           name=f"recfT{g}")
                            nc.vector.reciprocal_approx_fast(recf[0:1, :],
                                                             pvp[0:1, :])
                            rec = sb.tile([1, S], bf16, tag="rec", bufs=2,
                                          name=f"recT{g}")
                            with nc.allow_low_precision(
                                    reason="bf16 recip of sums"):
                                nc.vector.tensor_copy(rec[0:1, :],
                                                      recf[0:1, :])
                            recs[g] = rec
                        # broadcast 1/sum across partitions via K=1 matmuls
                        # (PE is idle here; skips the 2x2.1us serial Pool
                        # broadcasts on the tail critical path)
                        bcs = {}
                        for g, pvp in pair:
                            bcp = ps.tile([128, S], f32, tag="scp", bufs=2)
                            for n in range(2):
                                nc.tensor.matmul(
                                    bcp[0:HD1, n * 512:(n + 1) * 512],
                                    ones65[0:1, 0:HD1],
                                    recs[g][0:1, n * 512:(n + 1) * 512],
                                    start=True, stop=True)
                            bcs[g] = bcp
                        for g, pvp in pair:
                            pv_sb = sb.tile([HD1, S], bf16, tag="pvsb",
                                            bufs=4, name=f"pvsbT{g}")
                            nc.vector.tensor_copy(pv_sb[:], pvp[:])
                            ct = sb.tile([HD1, S], bf16, tag="ct", bufs=4,
                                         name=f"ctT{g}")
                            nc.vector.tensor_mul(ct[:], pv_sb[:],
                                                 bcs[g][0:HD1, :])
                            cts[(pss, g)] = ct
                        combine(cc, 0)
                        combine(cc, 1)
                    else:
                        norm(cc, pss, 0, pvp0)
                        if pss == 1:
                            combine(cc, 0)
                    if not inline_g1:
                        if prev_g1 is not None:
                            drain(prev_g1)
                        prev_g1 = gen_pv_sweep(cc, pss, 1, ess, pss == 1)
                        fillers.appendleft(prev_g1)
                if qk_next is not None:
                    drain(qk_next)

            drain_fillers()

            # ------------------------ output projection ---------------------
            # Pipelined: each group's cc0..6 partial accumulation is emitted
            # ahead of the previous group's cc7 finish, so the PE has ~6us of
            # work queued before the first matmul that needs the final
            # head-pair's attn_b (whose norm chain is still draining).
            ops = {}

            def o_finish(sc):
                op = ops.pop(sc)
                ob = sb.tile([128, DM], f32, tag="ob", bufs=2,
                             name=f"ob{sc}")
                for n in range(2):
                    nc.tensor.matmul(
                        op[:, n * 512:(n + 1) * 512],
                        attn_b[7][:, sc * 128:(sc + 1) * 128],
                        woT_sb[7][:, n * 512:(n + 1) * 512],
                        start=False, stop=True)
                for n in range(2):
                    nc.vector.tensor_copy(ob[:, n * 512:(n + 1) * 512],
                                          op[:, n * 512:(n + 1) * 512])
                    eng = nc.sync if n == 0 else nc.scalar
                    eng.dma_start(
                        out_d[sc * 128:(sc + 1) * 128,
                              n * 512:(n + 1) * 512],
                        ob[:, n * 512:(n + 1) * 512])

            for sc in range(8):
                op = ps.tile([128, DM], f32, tag="scp", bufs=2)
                ops[sc] = op
                for n in range(2):
                    for cc in range(7):
                        nc.tensor.matmul(
                            op[:, n * 512:(n + 1) * 512],
                            attn_b[cc][:, sc * 128:(sc + 1) * 128],
                            woT_sb[cc][:, n * 512:(n + 1) * 512],
                            start=(cc == 0), stop=False)
                if sc >= 1:
                    o_finish(sc - 1)
            o_finish(7)

    nc.compile()
    _CACHE[key] = nc
    return nc


def _prep_inputs(hidden_states, cos, sin, w_qkv, w_o):
    bf = ml_dtypes.bfloat16
    xT = np.ascontiguousarray(
        hidden_states.transpose(0, 2, 1)).astype(bf)          # [B, DM, S]
    wqkT = np.ascontiguousarray(w_qkv[:2 * DM].T)             # [DM, 2DM]
    # c-chunk-major repack: wqkr[t][p, dc*128+c] = wqkT[dc*128+p, t*128+c]
    wqkr = np.stack([
        np.ascontiguousarray(
            wqkT[:, t * 128:(t + 1) * 128]
            .reshape(8, 128, 128).transpose(1, 0, 2).reshape(128, DM))
        for t in range(16)]).astype(bf)                       # [16, 128, DM]
    wvT = np.ascontiguousarray(w_qkv[2 * DM:].T).astype(bf)   # [DM, DM]
    woT = np.ascontiguousarray(w_o.T).astype(bf)              # [DM, DM]

    idx = np.arange(S).reshape(32, 32).T.reshape(-1)
    d = np.arange(128) % HD
    sign = np.where(d < 32, -1.0, 1.0).astype(np.float32)
    trigC = np.stack([
        np.ascontiguousarray(cos[:, d].T),
        np.ascontiguousarray(cos[idx][:, d].T),
    ]).astype(bf)                                             # [2, 128, S]
    trigS = np.stack([
        np.ascontiguousarray(sin[:, d].T) * sign[:, None],
        np.ascontiguousarray(sin[idx][:, d].T) * sign[:, None],
    ]).astype(bf)
    shared = {"wqkr": wqkr, "wvT": wvT, "woT": woT,
              "trigC": trigC, "trigS": trigS}
    return [{"xT": np.ascontiguousarray(xT[b]), **shared} for b in range(B)]


def _install_ntff_hook():
    import sys, types
    if "antenv.axon_hooks" in sys.modules:
        return
    try:
        from trn_agent_boot.trn_boot import _ntff_profile_via_ctypes
        hook = _ntff_profile_via_ctypes('/opt/axon/libaxon_pjrt.so')
    except Exception:
        hook = None
    mod = types.ModuleType("antenv.axon_hooks")
    mod.get_axon_ntff_profile_hook = lambda: hook
    mod.set_axon_ntff_profile_hook = lambda h: None
    sys.modules["antenv.axon_hooks"] = mod


def kernel(hidden_states, cos, sin, w_qkv, w_o, _trace=False, _tmpdir=None):
    from concourse import bass_utils
    if _trace:
        _install_ntff_hook()
    nc = _build()
    in_maps = _prep_inputs(np.asarray(hidden_states, np.float32),
                           np.asarray(cos, np.float32),
                           np.asarray(sin, np.float32),
                           np.asarray(w_qkv, np.float32),
                           np.asarray(w_o, np.float32))
    res = bass_utils.run_bass_kernel_spmd(
        nc, in_maps, core_ids=list(range(NC)),
        trace=_trace, tmpdir=_tmpdir)
    out = np.stack([np.asarray(res.results[b]["out"], np.float32)
                    for b in range(B)])
    kernel.last_exec_time_ns = res.exec_time_ns
    return out


# revision 34
# speedup vs baseline: 1.1971x; 1.1971x over previous
"""Dual-RoPE attention block (B=8, S=1024, 16 heads x 64) on 8 NeuronCores.

Sharding: data-parallel over batch, one batch element per core.

Software-pipelined single-pass structure (v2): the per-head-pair attention
stream (scores -> exp -> PV) is ACT-paced; projection matmuls (QKV, woT
loads) are emitted as PE "filler" generators inside the attention stream so
the tensor engine never idles while the scalar engine exponentiates.

PSUM plan (8 banks): tag "scp" = 2 x [128,1024] f32 (4 banks), shared by
score tiles, projection n-group accumulators and o_proj accumulators -- every
allocation is a complete bank-group whose reader is emitted within the same
filler piece, so pool rotation WAR deps sequence reuse correctly.  Tag
"pvps" = 2 x [65,1024] f32 (4 banks): the inline head (g0) of the current
pass ping-pongs against the deferred second head (g1) sweep that overlaps
the next pass's units.

Numerics identical to v1 baseline: all matmuls bf16 in / fp32 PSUM, softmax
without max-subtraction, 1/sum via fast reciprocal of PV row 0
(vext col0 = 2.0 so rec = 0.5/sum includes the two-pass averaging).
"""

import numpy as np
import ml_dtypes
from collections import deque

B, S, DM = 8, 1024, 1024
NH, HD = 16, 64
HD1 = HD + 1
NC = 8                # cores

_CACHE = {}


def _build():
    key = "v2"
    if key in _CACHE:
        return _CACHE[key]
    from concourse import bacc, mybir
    import concourse.tile as tile

    f32 = mybir.dt.float32
    bf16 = mybir.dt.bfloat16
    EXP = mybir.ActivationFunctionType.Exp

    nc = bacc.Bacc("TRN2", target_bir_lowering=False, debug=False,
                   enable_asserts=False, num_devices=NC)

    xT_d = nc.dram_tensor("xT", [DM, S], bf16, kind="ExternalInput").ap()
    wqkr_d = nc.dram_tensor("wqkr", [16, 128, DM], bf16,
                            kind="ExternalInput").ap()
    wvT_d = nc.dram_tensor("wvT", [DM, DM], bf16, kind="ExternalInput").ap()
    woT_d = nc.dram_tensor("woT", [DM, DM], bf16, kind="ExternalInput").ap()
    trigC_d = nc.dram_tensor("trigC", [2, 128, S], bf16,
                             kind="ExternalInput").ap()
    trigS_d = nc.dram_tensor("trigS", [2, 128, S], bf16,
                             kind="ExternalInput").ap()
    out_d = nc.dram_tensor("out", [S, DM], f32, kind="ExternalOutput").ap()

    with tile.TileContext(nc) as tc:
        with (
            tc.tile_pool(name="sb", bufs=1) as sb,
            tc.tile_pool(name="ps", bufs=1, space="PSUM") as ps,
        ):
            # ---------------- persistent SBUF tiles + loads ----------------
            # Critical-path loads (first projection) go first on the sync
            # queue; everything else issues from the gpsimd queue so the
            # ~600ns-per-DMA descriptor-gen serialization doesn't delay the
            # first matmul.
            wqk0 = {}
            for t in (0, 8):
                w = sb.tile([128, DM], bf16, tag="wqk", bufs=6,
                            name=f"wqk{t}")
                nc.scalar.dma_start(w[:], wqkr_d[t])
                wqk0[t] = w

            xT_sb = [sb.tile([128, S], bf16, tag="xT", bufs=8, name=f"xT{i}")
                     for i in range(8)]
            for i in range(8):
                eng = nc.sync if i % 2 == 0 else nc.scalar
                eng.dma_start(xT_sb[i][:], xT_d[i * 128:(i + 1) * 128, :])

            trigC_t = [sb.tile([128, S], bf16, tag="trig", bufs=4,
                               name=f"trigC{p}") for p in range(2)]
            trigS_t = [sb.tile([128, S], bf16, tag="trig", bufs=4,
                               name=f"trigS{p}") for p in range(2)]
            for p in range(2):
                nc.sync.dma_start(trigC_t[p][:], trigC_d[p])
                nc.sync.dma_start(trigS_t[p][:], trigS_d[p])

            wvT_sb = [sb.tile([128, DM], bf16, tag="wv", bufs=8,
                              name=f"wv{i}") for i in range(8)]
            for i in range(8):
                nc.gpsimd.dma_start(wvT_sb[i][:],
                                    wvT_d[i * 128:(i + 1) * 128, :])

            ones65 = sb.tile([1, HD1], bf16, tag="ones", bufs=1,
                             name="ones65")
            nc.vector.memset(ones65[0:1, :], 1.0)

            vext = [sb.tile([128, NH * HD1], bf16, tag="vext", bufs=8,
                            name=f"vext{i}") for i in range(8)]
            attn_b = [sb.tile([128, S], bf16, tag="attnb", bufs=8,
                              name=f"attnb{i}") for i in range(8)]

            roped = {}   # (pss, t) -> tile; t: 0..7 q chunk, 8..15 k chunk
            cts = {}     # (pss, g) -> ct tile for the current cc
            woT_sb = []

            # ------------------- emission helper machinery -----------------
            fillers = deque()   # generators; each yield ~= one 8-mm piece

            def emit_filler(n=1):
                for _ in range(n):
                    while fillers:
                        try:
                            next(fillers[0])
                            break
                        except StopIteration:
                            fillers.popleft()
                    else:
                        return

            def drain(gen):
                for _ in gen:
                    pass

            def drain_fillers():
                while fillers:
                    drain(fillers.popleft())

            # ------------------------- generators --------------------------
            def gen_proj_qk(cc):
                """Project q (t=cc) and k (t=8+cc) into [c, s] layout, then
                RoPE: both chunks' matmul groups first, then pass-0 RoPE for
                both (DVE work pipelines under the k-chunk matmuls), pass-1
                RoPE last (consumed as early filler)."""
                chunks = (cc, 8 + cc)
                qks, sws = {}, {}
                for t in chunks:
                    if t in wqk0:
                        w = wqk0[t]
                    else:
                        w = sb.tile([128, DM], bf16, tag="wqk", bufs=6,
                                    name=f"wqk{t}")
                        nc.sync.dma_start(w[:], wqkr_d[t])
                    qk = sb.tile([128, S], bf16, tag="qk", bufs=2,
                                 name=f"qk{t}")
                    for n in range(2):
                        pst = ps.tile([128, S], f32, tag="scp", bufs=2)
                        for dc in range(8):
                            nc.tensor.matmul(
                                pst[:, 0:512],
                                w[:, dc * 128:(dc + 1) * 128],
                                xT_sb[dc][:, n * 512:(n + 1) * 512],
                                start=(dc == 0), stop=(dc == 7))
                        nc.vector.tensor_copy(qk[:, n * 512:(n + 1) * 512],
                                              pst[:, 0:512])
                        yield
                    sw = sb.tile([128, S], bf16, tag="sw", bufs=2,
                                 name=f"sw{t}")
                    for hh in range(2):
                        for f in range(2):
                            o0 = hh * 64 + f * 32
                            i0 = hh * 64 + (1 - f) * 32
                            nc.sync.dma_start(sw[o0:o0 + 32, :],
                                              qk[i0:i0 + 32, :])
                    qks[t], sws[t] = qk, sw
                for pss in range(2):
                    for t in chunks:
                        a = sb.tile([128, S], bf16, tag="ropeA", bufs=2,
                                    name=f"ropeA{t}_{pss}")
                        bb = sb.tile([128, S], bf16, tag="ropeB", bufs=2,
                                     name=f"ropeB{t}_{pss}")
                        r = sb.tile([128, S], bf16, tag="roped", bufs=12,
                                    name=f"rope{pss}_{t}")
                        nc.vector.tensor_mul(a[:], qks[t][:],
                                             trigC_t[pss][:])
                        nc.vector.tensor_mul(bb[:], sws[t][:],
                                             trigS_t[pss][:])
                        nc.vector.tensor_add(r[:], a[:], bb[:])
                        roped[(pss, t)] = r
                        yield

            def gen_proj_v():
                """V chunks in [s, c] row layout, strided into vext with 2.0
                in column 0 of each head block."""
                for sc in range(8):
                    vv = vext[sc][:].rearrange("p (h e) -> p h e", e=HD1)
                    nc.vector.memset(vv[:, :, 0:1], 2.0)
                    for n in range(2):
                        pst = ps.tile([128, S], f32, tag="scp", bufs=2)
                        for dc in range(8):
                            nc.tensor.matmul(
                                pst[:, 0:512],
                                xT_sb[dc][:, sc * 128:(sc + 1) * 128],
                                wvT_sb[dc][:, n * 512:(n + 1) * 512],
                                start=(dc == 0), stop=(dc == 7))
                        nc.vector.tensor_copy(
                            vv[:, 8 * n:8 * n + 8, 1:HD1],
                            pst[:, 0:512].rearrange("p (h e) -> p h e", e=HD))
                        yield

            def gen_wo():
                # woT reuses the wv SBUF slots; all V-proj reads of wvT are
                # emitted before this generator is reached in the queue, so
                # WAR deps sequence the overwrite correctly.
                for i in range(8):
                    w = sb.tile([128, DM], bf16, tag="wv", bufs=8,
                                name=f"wo{i}")
                    nc.sync.dma_start(w[:], woT_d[i * 128:(i + 1) * 128, :])
                    woT_sb.append(w)
                    if i % 4 == 3:
                        yield

            def norm(cc, pss, g, pvp):
                """ct = pv * (1/(2*sum)) broadcast across the 65 rows."""
                h = 2 * cc + g
                recf = sb.tile([1, S], f32, tag="recf", bufs=1,
                               name=f"recf{pss}_{h}")
                nc.vector.reciprocal_approx_fast(recf[0:1, :], pvp[0:1, :])
                rec = sb.tile([1, S], bf16, tag="rec", bufs=2,
                              name=f"rec{pss}_{h}")
                with nc.allow_low_precision(reason="bf16 recip of sums"):
                    nc.vector.tensor_copy(rec[0:1, :], recf[0:1, :])
                bc = sb.tile([HD1, S], bf16, tag="bc", bufs=2,
                             name=f"bc{pss}_{h}")
                nc.gpsimd.partition_broadcast(bc[:, :], rec[0:1, :],
                                              channels=HD1)
                # pv_sb cast runs on DVE while the broadcast runs on Pool
                pv_sb = sb.tile([HD1, S], bf16, tag="pvsb", bufs=4,
                                name=f"pvsb{pss}_{h}")
                nc.vector.tensor_copy(pv_sb[:], pvp[:])
                ct = sb.tile([HD1, S], bf16, tag="ct", bufs=4,
                             name=f"ct{pss}_{h}")
                nc.vector.tensor_mul(ct[:], pv_sb[:], bc[:])
                cts[(pss, g)] = ct

            def combine(cc, g):
                ah = sb.tile([HD1, S], bf16, tag="ah", bufs=2, name=f"ah{g}")
                nc.vector.tensor_add(ah[:], cts[(0, g)][:], cts[(1, g)][:])
                eng = nc.sync if g == 0 else nc.scalar
                eng.dma_start(attn_b[cc][g * 64:(g + 1) * 64, :],
                              ah[1:HD1, :])

            def gen_pv_sweep(cc, pss, g, ess, do_combine):
                """Deferred PV sweep for head g of (cc, pss), then its norm
                (and the head-pair combine when this is the last piece)."""
                h = 2 * cc + g
                pvp = ps.tile([HD1, S], f32, tag="pvps", bufs=2)
                for kc in range(8):
                    for n in range(2):
                        nc.tensor.matmul(
                            pvp[:, n * 512:(n + 1) * 512],
                            vext[kc][:, h * HD1:(h + 1) * HD1],
                            ess[(kc, n)][:, g * 512:(g + 1) * 512],
                            start=(kc == 0), stop=(kc == 7))
                    if kc % 2 == 1:
                        yield
                norm(cc, pss, g, pvp)
                if do_combine:
                    combine(cc, g)
                yield

            # --------------------------- schedule ---------------------------
            # Lead-in: head-pair 0's four matmul groups + pass-0 RoPE run
            # inline (nothing else in the DVE queue so the rope chain fires
            # as the groups complete); pass-1 RoPE pieces lead the filler
            # queue, followed by V-proj and woT loads.
            g0 = gen_proj_qk(0)
            for _ in range(6):
                next(g0)
            fillers.append(g0)
            v_gen = gen_proj_v()
            fillers.append(v_gen)
            fillers.append(gen_wo())

            prev_g1 = None
            for cc in range(8):
                hE = 2 * cc
                qk_next = None
                if cc < 7:
                    qk_next = gen_proj_qk(cc + 1)
                    fillers.append(qk_next)
                for pss in range(2):
                    while (pss, cc) not in roped or (pss, 8 + cc) not in roped:
                        emit_filler(1)
                    q1 = {pss: roped[(pss, cc)]}
                    k1 = {pss: roped[(pss, 8 + cc)]}
                    defer_g0 = (cc == 0 and pss == 0)
                    if defer_g0:
                        # put a few V pieces ahead of the first scores in the
                        # PE queue: they execute while the RoPE chain finishes
                        emit_filler(6)
                    inline_g1 = (cc == 7 and pss == 1)
                    ess = {}
                    if inline_g1 and prev_g1 is not None:
                        # Pre-drain the previous deferred sweep so the final
                        # head-pair's tail chain is as short as possible.
                        drain(prev_g1)
                        prev_g1 = None
                    if not defer_g0:
                        pvp0 = ps.tile([HD1, S], f32, tag="pvps", bufs=2)
                    if inline_g1:
                        pvp1 = ps.tile([HD1, S], f32, tag="pvps", bufs=2)
                    for kc in range(8):
                        for n in range(2):
                            scp = ps.tile([128, S], f32, tag="scp", bufs=2)
                            for g, hh in ((0, 0), (1, 64)):
                                nc.tensor.matmul(
                                    scp[:, g * 512:(g + 1) * 512],
                                    k1[pss][hh:hh + 64,
                                            kc * 128:(kc + 1) * 128],
                                    q1[pss][hh:hh + 64,
                                            n * 512:(n + 1) * 512],
                                    start=True, stop=True)
                            es = sb.tile([128, S], bf16, tag="es", bufs=20,
                                         name=f"es{pss}_{cc}_{kc}_{n}")
                            nc.scalar.activation(es[:], scp[:], EXP,
                                                 scale=0.125)
                            ess[(kc, n)] = es
                            emit_filler(1)
                            if not defer_g0:
                                nc.tensor.matmul(
                                    pvp0[:, n * 512:(n + 1) * 512],
                                    vext[kc][:, hE * HD1:(hE + 1) * HD1],
                                    es[:, 0:512],
                                    start=(kc == 0), stop=(kc == 7))
                            if inline_g1:
                                nc.tensor.matmul(
                                    pvp1[:, n * 512:(n + 1) * 512],
                                    vext[kc][:, (hE + 1) * HD1:
                                               (hE + 2) * HD1],
                                    es[:, 512:1024],
                                    start=(kc == 0), stop=(kc == 7))
                    if defer_g0:
                        # vext must be fully projected before any PV of
                        # head-pair 0; drain V then run g0's sweep inline.
                        drain(v_gen)
                        drain(gen_pv_sweep(cc, pss, 0, ess, False))
                    elif inline_g1:
                        # final head-pair: interleave the two norm chains for
                        # minimum latency before o_proj can start
                        pair = ((0, pvp0), (1, pvp1))
                        recs = {}
                        for g, pvp in pair:
                            recf = sb.tile([1, S], f32, tag="recf", bufs=1,
                                           name=f"recfT{g}")
                            nc.vector.reciprocal_approx_fast(recf[0:1, :],
                                                             pvp[0:1, :])
                            rec = sb.tile([1, S], bf16, tag="rec", bufs=2,
                                          name=f"recT{g}")
                            with nc.allow_low_precision(
                                    reason="bf16 recip of sums"):
                                nc.vector.tensor_copy(rec[0:1, :],
                                                      recf[0:1, :])
                            recs[g] = rec
                        bcs = {}
                        for g, pvp in pair:
                            bc = sb.tile([HD1, S], bf16, tag="bc", bufs=2,
                                         name=f"bcT{g}")
                            nc.gpsimd.partition_broadcast(
                                bc[:, :], recs[g][0:1, :], channels=HD1)
                            bcs[g] = bc
                        for g, pvp in pair:
                            pv_sb = sb.tile([HD1, S], bf16, tag="pvsb",
                                            bufs=4, name=f"pvsbT{g}")
                            nc.vector.tensor_copy(pv_sb[:], pvp[:])
                            ct = sb.tile([HD1, S], bf16, tag="ct", bufs=4,
                                         name=f"ctT{g}")
                            nc.vector.tensor_mul(ct[:], pv_sb[:], bcs[g][:])
                            cts[(pss, g)] = ct
                        combine(cc, 0)
                        combine(cc, 1)
                    else:
                        norm(cc, pss, 0, pvp0)
                        if pss == 1:
                            combine(cc, 0)
                    if not inline_g1:
                        if prev_g1 is not None:
                            drain(prev_g1)
                        prev_g1 = gen_pv_sweep(cc, pss, 1, ess, pss == 1)
                        fillers.appendleft(prev_g1)
                if qk_next is not None:
                    drain(qk_next)

            drain_fillers()

            # ------------------------ output projection ---------------------
            # Pipelined: each group's cc0..6 partial accumulation is emitted
            # ahead of the previous group's cc7 finish, so the PE has ~6us of
            # work queued before the first matmul that needs the final
            # head-pair's attn_b (whose norm chain is still draining).
            ops = {}

            def o_finish(sc):
                op = ops.pop(sc)
                ob = sb.tile([128, DM], f32, tag="ob", bufs=2,
                             name=f"ob{sc}")
                for n in range(2):
                    nc.tensor.matmul(
                        op[:, n * 512:(n + 1) * 512],
                        attn_b[7][:, sc * 128:(sc + 1) * 128],
                        woT_sb[7][:, n * 512:(n + 1) * 512],
                        start=False, stop=True)
                for n in range(2):
                    nc.vector.tensor_copy(ob[:, n * 512:(n + 1) * 512],
                                          op[:, n * 512:(n + 1) * 512])
                    eng = nc.sync if n == 0 else nc.scalar
                    eng.dma_start(
                        out_d[sc * 128:(sc + 1) * 128,
                              n * 512:(n + 1) * 512],
                        ob[:, n * 512:(n + 1) * 512])

            for sc in range(8):
                op = ps.tile([128, DM], f32, tag="scp", bufs=2)
                ops[sc] = op
                for n in range(2):
                    for cc in range(7):
                        nc.tensor.matmul(
                            op[:, n * 512:(n + 1) * 512],
                            attn_b[cc][:, sc * 128:(sc + 1) * 128],
                            woT_sb[cc][:, n * 512:(n + 1) * 512],
                            start=(cc == 0), stop=False)
                if sc >= 1:
                    o_finish(sc - 1)
            o_finish(7)

    nc.compile()
    _CACHE[key] = nc
    return nc


def _prep_inputs(hidden_states, cos, sin, w_qkv, w_o):
    bf = ml_dtypes.bfloat16
    xT = np.ascontiguousarray(
        hidden_states.transpose(0, 2, 1)).astype(bf)          # [B, DM, S]
    wqkT = np.ascontiguousarray(w_qkv[:2 * DM].T)             # [DM, 2DM]
    # c-chunk-major repack: wqkr[t][p, dc*128+c] = wqkT[dc*128+p, t*128+c]
    wqkr = np.stack([
        np.ascontiguousarray(
            wqkT[:, t * 128:(t + 1) * 128]
            .reshape(8, 128, 128).transpose(1, 0, 2).reshape(128, DM))
        for t in range(16)]).astype(bf)                       # [16, 128, DM]
    wvT = np.ascontiguousarray(w_qkv[2 * DM:].T).astype(bf)   # [DM, DM]
    woT = np.ascontiguousarray(w_o.T).astype(bf)              # [DM, DM]

    idx = np.arange(S).reshape(32, 32).T.reshape(-1)
    d = np.arange(128) % HD
    sign = np.where(d < 32, -1.0, 1.0).astype(np.float32)
    trigC = np.stack([
        np.ascontiguousarray(cos[:, d].T),
        np.ascontiguousarray(cos[idx][:, d].T),
    ]).astype(bf)                                             # [2, 128, S]
    trigS = np.stack([
        np.ascontiguousarray(sin[:, d].T) * sign[:, None],
        np.ascontiguousarray(sin[idx][:, d].T) * sign[:, None],
    ]).astype(bf)
    shared = {"wqkr": wqkr, "wvT": wvT, "woT": woT,
              "trigC": trigC, "trigS": trigS}
    return [{"xT": np.ascontiguousarray(xT[b]), **shared} for b in range(B)]


def _install_ntff_hook():
    import sys, types
    if "antenv.axon_hooks" in sys.modules:
        return
    try:
        from trn_agent_boot.trn_boot import _ntff_profile_via_ctypes
        hook = _ntff_profile_via_ctypes('/opt/axon/libaxon_pjrt.so')
    except Exception:
        hook = None
    mod = types.ModuleType("antenv.axon_hooks")
    mod.get_axon_ntff_profile_hook = lambda: hook
    mod.set_axon_ntff_profile_hook = lambda h: None
    sys.modules["antenv.axon_hooks"] = mod


def kernel(hidden_states, cos, sin, w_qkv, w_o, _trace=False, _tmpdir=None):
    from concourse import bass_utils
    if _trace:
        _install_ntff_hook()
    nc = _build()
    in_maps = _prep_inputs(np.asarray(hidden_states, np.float32),
                           np.asarray(cos, np.float32),
                           np.asarray(sin, np.float32),
                           np.asarray(w_qkv, np.float32),
                           np.asarray(w_o, np.float32))
    res = bass_utils.run_bass_kernel_spmd(
        nc, in_maps, core_ids=list(range(NC)),
        trace=_trace, tmpdir=_tmpdir)
    out = np.stack([np.asarray(res.results[b]["out"], np.float32)
                    for b in range(B)])
    kernel.last_exec_time_ns = res.exec_time_ns
    return out
